# revision 49
# baseline (speedup 1.0000x reference)
"""Distributed Bass kernel for nn_Attention_57612691309274 on 8 TRN2 NeuronCores.

Reference computes, for x [B=2, S=2048, D=1024], H=16 heads, Dh=64:
  q/k/v = einsum('bsd,hde->bshe', x, W) + b, scaled by 1/sqrt(D)
  scores = q@k^T / sqrt(Dh), causal mask, softmax
  out = ((softmax @ v) @ W_O) * 1/sqrt(Dh) + b_O

Sharding: core c => batch b = c//4, head-group hg = c%4 (heads 4hg..4hg+3).
Each core projects q/k/v for its 4 heads over its batch and runs causal
attention in a [k, q]-transposed score layout (softmax needs no partition
reduction; umup scaling keeps scores ~N(0,1) so no max subtraction).

Key structure:
 - AV matmul stationary per head is [ones(64 cols) | v(64 cols)]: PSUM rows
   0-63 hold the softmax denominator replicated across 64 partitions BY THE
   PE (free: matmul cost is moving-column bound), rows 64-127 the
   unnormalized z. Normalization = reciprocal_approx_fast on rows 0-63
   (base partition 0 — the custom DVE op reads its input at the OUTPUT's
   base partition, so in/out bases must match) + one multiply reading the
   PSUM z rows at base 64 (PSUM operands carry their own base field; two
   SBUF inputs would have to share one). No DRAM round trips and the exact
   reciprocal (6 cyc/elem) is avoided.
 - Per head pair, z accumulates in two single-bank PSUM tiles (one per
   head) so the next pair's accumulation only waits on one head's
   normalize, not the whole chain.
 - Head pairs (even head on PE rows 0-63, odd on 64-127) issue score
   matmuls to disjoint row-groups that run concurrently and share one exp.
   The two heads' scores always land in separate PSUM banks (concurrent
   row-group matmuls must not share a bank).
 - Emission is software-pipelined and interleaved: score(t+1) is emitted
   before AV(t), and qkv-projection / out-projection "filler" chunks are
   spread between attention tiles so the PE never idles while exp runs and
   the scalar engine is fed as early as possible. Filler chunks are
   self-contained (a PSUM tile held across a filler gap deadlocks the
   round-robin pool).
 - Input loads are phased: the first q-projection's operands (wq + xT
   block 0, chunked per 128 columns in consumption order) go out first
   across the sync/gpsimd/scalar queues; everything else queues behind.
 - Output combine: instead of projecting the local 256 head-dims to the
   full 1024 model dims and ReduceScatter-summing 4 MB of partials (the
   old scheme: ~137us of serial CC-stream time, ~57us of it a pure tail
   with all compute engines idle), each query block's z^T (256 dims x 512
   rows, bf16) is exchanged via ONE 8-core AllToAll (256 KB, measured
   6.5-8.2us on the Mesh algorithm): core g receives, for ITS 64 rows per
   batch of that block, all 1024 z-dims (256 from each same-batch peer),
   then applies the FULL W_O (all 16 heads) locally with fp32 PSUM
   accumulation over the whole 1024-dim contraction. Same PE cycles, 3.4x
   less wire, and the reduction disappears into the exchange.
 - A 1 KB warmup AllToAll is triggered at kernel start so the
   mesh/communicator setup and first-op peer sync are absorbed while the
   collective stream is otherwise idle (the post-barrier window).
 - Core c owns, per 512-row block j and batch b, query rows
   512j+64c..+64; its out tensor rows are (j, b, r) -> 128j+64b+r.
 - Block 3 is exchanged as two per-e-tile 128KB AllToAlls: the et=0 half
   only needs head-pair 0's normalize and fires ~half a stream early; the
   tail interleaves final-op(2) halves around block-3's even/odd dim
   chunks so the PE has real work during both half-exchange waits.
 - Startup: only the m0 halves of q(0)/k(0) + v-tiles 0-1 precompute; the
   m1 halves and v2/v3 ride as stream-0 fillers during head-pair 0 (they
   MUST fire before head-pair 1 — the PE queue is FIFO and a late filler
   behind hp1's scores deadlocks). wo/bo/xT blk2+3 are deferred past the
   startup DMA crunch (the three DMA queues drain concurrently and share
   the HBM port).

Timing context (axon 8-core env): all 8 cores are on one chip; PJRT
launches them with random 0-90us skew, which every 8-core collective
inherits — exec variance is ~±40us run to run, best reps ~200-207us vs
the 230-242us ReduceScatter baseline. The compute span itself (~150us)
is PE-clock-bound (GPIO throttle 13/16).

All umup scale factors are folded on the host into W_Q (1/8192) and W_O
(1/256). x is fed pre-transposed and pre-cast to bf16.
"""

import os
import sys

if "/opt/trn_rl_repo" not in sys.path:
    sys.path.insert(0, "/opt/trn_rl_repo")

import numpy as np
import ml_dtypes

import concourse.bass as bass
import concourse.tile as tile
from concourse import bacc, mybir

BF16 = np.dtype(ml_dtypes.bfloat16)
F32 = np.float32

B, S, D, H, DH = 2, 2048, 1024, 16, 64
HC = 4            # heads per core
E = HC * DH       # 256 head-dim columns per core
N_CORES = 8
CORE_IDS = list(range(N_CORES))
QB = 512          # full query block
P = 128

_NC_CACHE = {}


def build_kernel():
    nc = bacc.Bacc("TRN2", target_bir_lowering=False, debug=False,
                   num_devices=N_CORES)
    dt = mybir.dt

    # ---- external I/O (per-core shards fed from host) ----
    xT_d = nc.dram_tensor("xT", [D, S], dt.bfloat16, kind="ExternalInput")
    wq_d = nc.dram_tensor("wq", [D, E], dt.bfloat16, kind="ExternalInput")
    wk_d = nc.dram_tensor("wk", [D, E], dt.bfloat16, kind="ExternalInput")
    wv_d = nc.dram_tensor("wv", [D, E], dt.bfloat16, kind="ExternalInput")
    wo_d = nc.dram_tensor("wo", [D, D], dt.bfloat16, kind="ExternalInput")
    bq_d = nc.dram_tensor("bq", [P, 2], dt.float32, kind="ExternalInput")
    bk_d = nc.dram_tensor("bk", [P, 2], dt.float32, kind="ExternalInput")
    bv_d = nc.dram_tensor("bv", [P, E], dt.float32, kind="ExternalInput")
    bo_d = nc.dram_tensor("bo", [1, D], dt.float32, kind="ExternalInput")
    tri_d = nc.dram_tensor("tri", [P, P], dt.bfloat16, kind="ExternalInput")
    # out rows: 128j + 64b + r  <->  (batch b, seq 512j + 64*core + r)
    out_d = nc.dram_tensor("out", [QB, D], dt.bfloat16, kind="ExternalOutput")

    # ---- internal DRAM (AllToAll staging; collectives can't touch I/O) ----
    # az row layout: dest/src-major: row = g*256 + et*128 + p  (dim
    # 256*hg + 128*et + p of the sender), col = query row r within the
    # 64-row slice owned by dest g.
    az_in = [nc.dram_tensor(f"az_in{j}", [8 * E, 64], dt.bfloat16)
             for j in range(3)]
    az_out = [nc.dram_tensor(f"az_out{j}", [8 * E, 64], dt.bfloat16)
              for j in range(3)]
    # block 3 exchanged as two per-e-tile AllToAlls: the et=0 half only
    # needs head-pair 0's normalize and fires half a stream earlier
    az3_in = [nc.dram_tensor(f"az3_in{et}", [8 * P, 64], dt.bfloat16)
              for et in range(2)]
    az3_out = [nc.dram_tensor(f"az3_out{et}", [8 * P, 64], dt.bfloat16)
               for et in range(2)]
    wu_in = nc.dram_tensor("wu_in", [8, 64], dt.bfloat16)
    wu_out = nc.dram_tensor("wu_out", [8, 64], dt.bfloat16)

    a2a_group = [list(range(8))]
    Exp = mybir.ActivationFunctionType.Exp
    ADD = mybir.AluOpType.add
    MUL = mybir.AluOpType.mult
    BYP = mybir.AluOpType.bypass

    with tile.TileContext(nc) as tc:
        with (
            tc.tile_pool(name="persist", bufs=1) as pp,
            tc.tile_pool(name="etile", bufs=8) as ep,
            tc.tile_pool(name="obuf", bufs=2) as op_,
            tc.tile_pool(name="rcp", bufs=2) as rp,
            tc.tile_pool(name="sc", bufs=3, space="PSUM") as scp,
            tc.tile_pool(name="zp", bufs=2, space="PSUM") as zpp,
        ):
            # ---------- staged input loads ----------
            # Phase 1 (everything the first ~25us of compute needs, and
            # nothing else, so the shared DMA bandwidth all goes to it):
            # wq + xT block 0 on sync, wk/wv on scalar. Later blocks and
            # constants are emitted behind them.
            xT = pp.tile([P, 8, S], dt.bfloat16, tag="xT")
            xT_v = xT_d.ap().rearrange("(o p) f -> p o f", p=P)
            wq = pp.tile([P, 8, E], dt.bfloat16, tag="wq")
            wq_v = wq_d.ap().rearrange("(o p) f -> p o f", p=P)
            wk = pp.tile([P, 8, E], dt.bfloat16, tag="wk")
            wk_v = wk_d.ap().rearrange("(o p) f -> p o f", p=P)
            # Warmup collective first: the AllToAll mesh setup + first-op
            # peer sync run on the otherwise-idle CC stream while compute
            # streams from DMA (the CC stream can't start before the device
            # barrier ends ~50-70us in anyway).
            nc.gpsimd.dma_start(wu_in.ap()[:], xT_d.ap()[0:8, 0:64])
            nc.gpsimd.collective_compute(
                "AllToAll", BYP, replica_groups=a2a_group,
                ins=[wu_in.ap().opt()], outs=[wu_out.ap().opt()])
            # Tiny consts go FIRST: DMA-completion semaphores are a small
            # reused pool assigned in emission order, and a consumer waits on
            # its dep-DMA's (sem, threshold) — if a const shares a sem slot
            # behind a 1MB transfer, its consumer falsely waits on that
            # transfer (measured: 10us vector stall -> PE idle -> HAM cold).
            bq = pp.tile([P, 2], dt.float32, tag="bq")
            nc.sync.dma_start(bq[:], bq_d.ap()[:])
            bk = pp.tile([P, 2], dt.float32, tag="bk")
            nc.sync.dma_start(bk[:], bk_d.ap()[:])
            tri = pp.tile([P, P], dt.bfloat16, tag="tri")
            nc.sync.dma_start(tri[:], tri_d.ap()[:])
            bv = pp.tile([P, E], dt.float32, tag="bv")
            nc.gpsimd.dma_start(bv[:], bv_d.ap()[:])
            # chunked in the order the first q-projection consumes them, and
            # with the t=0 matmul's two operands (wq chunk, xT chunk) leading
            # DIFFERENT queues so they transfer in parallel: the first matmul
            # waits on ~200KB landing concurrently, not serially. (Only
            # sync/gpsimd/scalar queues can issue DMAs.)
            # wv chunks ride the same per-t loop (on sync, behind each wq
            # chunk): a single trailing 1MB wv DMA lands only ~35us in and
            # stalls the v-tiles (and with them stream 0) for ~12us.
            wv = pp.tile([P, 8, E], dt.bfloat16, tag="wv")
            wv_v = wv_d.ap().rearrange("(o p) f -> p o f", p=P)
            for t in range(8):
                # (xT chunks alternating gpsimd/sync was tried and measured
                # neutral-to-worse: it un-dilutes xT but dilutes wq, and
                # the m0 chains gate on both)
                nc.gpsimd.dma_start(xT[:, t:t + 1, 0:QB], xT_v[:, t:t + 1, 0:QB])
                nc.sync.dma_start(wq[:, t:t + 1], wq_v[:, t:t + 1])
                nc.scalar.dma_start(wk[:, t:t + 1], wk_v[:, t:t + 1])
            # wv rides scalar BEHIND all wk chunks: interleaving it into
            # the per-t loop delays the q/k m0 chains (measured +6us of
            # startup gaps); the wv wait is hidden by the m1 halves below
            nc.scalar.dma_start(wv[:], wv_v[:])
            # Phase 2: remaining xT blocks + wo. Block 1 rides the scalar
            # queue — it carries only the 0.5MB of wk chunks, so b1 lands
            # ~22-28us, before j=0's qkv(1) filler matmuls need it. (Putting
            # b1 on gpsimd behind the 1MB block-0 load was measured worse:
            # it arrives ~39us and stalls the tensor stream at the fillers.)
            nc.scalar.dma_start(xT[:, :, QB:2 * QB], xT_v[:, :, QB:2 * QB])
            # wo (2MB full W_O) + bo + xT blk2/3 are deferred until after
            # stream 0 (emitted there, below): all three queues drain their
            # DMA lists eagerly and concurrently, so anything emitted here
            # competes with wv/xT0/xT1 at the shared HBM port during the
            # startup crunch (measured: wv +15-20us late -> PE stall).
            bo = pp.tile([P, D], dt.float32, tag="bo")
            wo = pp.tile([P, 8, D], dt.bfloat16, tag="wo")

            qT = pp.tile([P, 2, S], dt.bfloat16, tag="qT")
            kT = pp.tile([P, 2, S], dt.bfloat16, tag="kT")
            # per head h: cols [128h, 128h+64) = 1.0, [128h+64, 128h+128) = v
            # (AV output rows 0-63 = softmax denominator replicated by the PE
            # at base partition 0 — required by reciprocal_approx_fast —
            # rows 64-127 = unnormalized z)
            vsb = pp.tile([P, 16, HC * P], dt.bfloat16, tag="vsb")
            # on VECTOR (idle at start): a gpsimd memset blocks the gpsimd
            # queue's xT chunk DMA issue for ~3.5us
            nc.vector.memset(
                vsb.rearrange("p t (h c) -> p t h c", h=HC)[:, :, :, 0:DH], 1.0)
            # z^T staging, laid out [p, e-tile(2), q-block(4), 512]
            zT = pp.tile([P, 2, 4, QB], dt.bfloat16, tag="zT")
            # received z for my rows: [p, block j, dim-chunk t(8), batch, 64]
            # (p, t) <-> global z-dim 128t + p; (batch, r) <-> psum out row
            zrecv = pp.tile([P, 3, 8, 2, 64], dt.bfloat16, tag="zrecv")
            # block 3 arrives per-e-tile: chunk (et, sf) <-> t = 2*sf + et
            zrecv3 = pp.tile([P, 2, 4, 2, 64], dt.bfloat16, tag="zrecv3")

            # ---------- filler chunk emitters (qkv projections / outproj) ----
            def emit_qk_half(jb, which, m):
                """One m-half (8 MMs + bias add) of a q/k projection for
                block jb. Self-contained: the PSUM tile is released before
                the next chunk (holding it across filler gaps deadlocks the
                round-robin pool)."""
                w_sb, b_sb, dst = {
                    "q": (wq, bq, qT), "k": (wk, bk, kT)}[which]
                ps = scp.tile([P, 2, QB], dt.float32, tag="sc", name="qkps")
                for t in range(8):
                    nc.tensor.matmul(
                        ps[:, m, :],
                        lhsT=w_sb[:, t, P * m:P * (m + 1)],
                        rhs=xT[:, t, QB * jb:QB * (jb + 1)],
                        start=(t == 0), stop=(t == 7),
                    )
                nc.vector.tensor_tensor(
                    out=dst[:, m, QB * jb:QB * (jb + 1)],
                    in0=ps[:, m, :],
                    in1=b_sb[:, m, None].to_broadcast([P, QB]),
                    op=ADD,
                )

            def emit_v_tile(jt):
                """v projection for one 128-row tile of the sequence."""
                ps = scp.tile([P, 2, QB], dt.float32, tag="sc")
                psv = ps[:, 0, :E]
                for t in range(8):
                    nc.tensor.matmul(
                        psv,
                        lhsT=xT[:, t, P * jt:P * (jt + 1)],
                        rhs=wv[:, t, :],
                        start=(t == 0), stop=(t == 7),
                    )
                nc.vector.tensor_tensor(
                    out=vsb[:, jt].rearrange(
                        "p (h c) -> p h c", h=HC)[:, :, DH:P],
                    in0=psv.rearrange("p (h e) -> p h e", h=HC),
                    in1=bv.rearrange("p (h e) -> p h e", h=HC),
                    op=ADD,
                )

            def emit_a2a(j):
                """Stage block j's z^T to DRAM and fire the 8-core AllToAll.
                Dest g's slice = my z for q rows 64g..64g+64 of the block;
                row g*256 + et*128 + p carries my z-dim (et, p)."""
                av = az_in[j].ap().rearrange("(g et p) r -> p g et r",
                                             g=8, et=2, p=P)
                for et in range(2):
                    nc.sync.dma_start(
                        av[:, :, et, :],
                        zT[:, et, j, :].rearrange("p (g r) -> p g r", g=8))
                nc.gpsimd.collective_compute(
                    "AllToAll", BYP, replica_groups=a2a_group,
                    ins=[az_in[j].ap().opt()], outs=[az_out[j].ap().opt()])
                # eager readback: out row bh*1024 + sf*256 + et*128 + p is
                # src (bh*4+sf)'s z-dim 256*sf+128*et+p for my row r of
                # batch bh -> zrecv[p, j, t=(sf,et), bh, r]
                for bh, q in ((0, nc.sync), (1, nc.gpsimd)):
                    q.dma_start(
                        zrecv[:, j, :, bh, :],
                        az_out[j].ap()[1024 * bh:1024 * (bh + 1), :]
                        .rearrange("(t p) r -> p t r", t=8, p=P))

            def emit_a2a3(et):
                """Block-3 half exchange for e-tile et ([1024, 64], 128KB).
                Output row (bh*4+sf)*128 + p -> zrecv3 chunk (et, sf)."""
                nc.sync.dma_start(
                    az3_in[et].ap().rearrange("(g p) r -> p g r", g=8, p=P),
                    zT[:, et, 3, :].rearrange("p (g r) -> p g r", g=8))
                nc.gpsimd.collective_compute(
                    "AllToAll", BYP, replica_groups=a2a_group,
                    ins=[az3_in[et].ap().opt()], outs=[az3_out[et].ap().opt()])
                # per-(bh, sf-half) loads spread over three queues: these
                # striped transfers run at ~128B-line efficiency, and the
                # first dim-chunks gate the tail matmuls — smaller pieces
                # land the first lhsT slab ~2us sooner
                qs = [nc.sync, nc.gpsimd, nc.scalar, nc.sync]
                for i, (bh, s2) in enumerate(
                        ((0, 0), (1, 0), (0, 1), (1, 1))):
                    qs[i].dma_start(
                        zrecv3[:, et, 2 * s2:2 * (s2 + 1), bh, :],
                        az3_out[et].ap()[512 * bh + 256 * s2:
                                         512 * bh + 256 * (s2 + 1), :]
                        .rearrange("(sf p) r -> p sf r", sf=2, p=P))

            def emit_tail():
                """Tail: final-op(2) interleaved with final-op(3) so the PE
                has real work during both half-exchange waits — fo2's nb=0
                covers the et=0 wait, fo2's nb=1 covers et=1's wire time,
                and block 3's even chunks run between them."""
                bo2 = bo.rearrange("p (n f) -> p n f", n=2)
                ps2 = scp.tile([P, 2, QB], dt.float32, tag="sc", name="ps2")
                ps3 = scp.tile([P, 2, QB], dt.float32, tag="sc", name="ps3")

                def fo2_half(nb):
                    for t in range(8):
                        nc.tensor.matmul(
                            ps2[:, nb, :],
                            lhsT=zrecv[:, 2, t].rearrange("p b r -> p (b r)"),
                            rhs=wo[:, t, QB * nb:QB * (nb + 1)],
                            start=(t == 0), stop=(t == 7),
                        )

                def fo3_half(et):
                    # sf-outer: both nb matmuls share one stationary load
                    for sf in range(4):
                        for nb in range(2):
                            nc.tensor.matmul(
                                ps3[:, nb, :],
                                lhsT=zrecv3[:, et, sf].rearrange(
                                    "p b r -> p (b r)"),
                                rhs=wo[:, 2 * sf + et, QB * nb:QB * (nb + 1)],
                                start=(et == 0 and sf == 0),
                                stop=(et == 1 and sf == 3),
                            )

                def keep_warm(n):
                    # harmless matmuls into a scratch bank: the et1 wait is
                    # ~5us, long enough for the HAM MID window to
                    # re-throttle the PE to 1.2GHz right before the real
                    # tail matmuls. lhsT MUST be et0-exchange data: with a
                    # dep-free operand (wo, loaded ~50us) the scheduler
                    # hoists these into the PE-bound middle, where they
                    # cost ~2.6us and warm nothing.
                    scr = zpp.tile([P, QB], dt.float32, tag="z", name="scr")
                    for i in range(n):
                        nc.tensor.matmul(
                            scr[:, :],
                            lhsT=zrecv3[:, 0, i % 4].rearrange(
                                "p b r -> p (b r)"),
                            rhs=wo[:, i, 0:QB],
                            start=True, stop=True,
                            skip_group_check=True,
                        )

                emit_final_op(1)   # real work covering the et0-zrecv wait
                fo2_half(0)
                fo3_half(0)
                fo2_half(1)
                keep_warm(6)
                ob2 = op_.tile([P, 2, QB], dt.bfloat16, tag="opb", name="ob2")
                nc.vector.tensor_tensor(
                    out=ob2[:], in0=ps2[:], in1=bo2, op=ADD)
                nc.gpsimd.dma_start(
                    out_d.ap()[2 * P:3 * P, :].rearrange(
                        "p (n f) -> p n f", n=2),
                    ob2[:])
                fo3_half(1)
                # bias + writeout split per nb half across queues: the
                # first half's out-DMA overlaps the second half's bias add
                ob = op_.tile([P, 2, QB], dt.bfloat16, tag="opb", name="ob3")
                for nbh, q in ((0, nc.sync), (1, nc.gpsimd)):
                    nc.vector.tensor_tensor(
                        out=ob[:, nbh], in0=ps3[:, nbh],
                        in1=bo2[:, nbh], op=ADD)
                    q.dma_start(
                        out_d.ap()[3 * P:4 * P, QB * nbh:QB * (nbh + 1)],
                        ob[:, nbh])

            def emit_final_op(j):
                """Full out-projection for my 128 output rows of block j
                (64 rows per batch): contraction over all 1024 z-dims in 8
                chunks, fp32 PSUM accumulation, + full bias, DMA to out."""
                ps = scp.tile([P, 2, QB], dt.float32, tag="sc")
                # t-outer: both nb matmuls share one stationary load
                for t in range(8):
                    for nb in range(2):
                        nc.tensor.matmul(
                            ps[:, nb, :],
                            lhsT=zrecv[:, j, t].rearrange("p b r -> p (b r)"),
                            rhs=wo[:, t, QB * nb:QB * (nb + 1)],
                            start=(t == 0), stop=(t == 7),
                        )
                ob = op_.tile([P, 2, QB], dt.bfloat16, tag="opb")
                nc.vector.tensor_tensor(
                    out=ob[:], in0=ps[:],
                    in1=bo.rearrange("p (n f) -> p n f", n=2), op=ADD)
                nc.gpsimd.dma_start(
                    out_d.ap()[P * j:P * (j + 1), :].rearrange(
                        "p (n f) -> p n f", n=2),
                    ob[:])

            # ---------- attention stream ----------
            def attention_stream(j, q0, nq, n_kt, zq0, fillers, sched=None):
                """Causal attention for query rows [q0, q0+nq) of block j,
                over n_kt 128-row k-tiles, for both head pairs. Emission is
                software-pipelined (score(t+1) before AV(t)) and filler
                chunks are spread across AV points; any not reached are
                flushed at the end of the stream."""
                n_f = len(fillers)
                if sched is None:
                    sched = [(i + 1) * (2 * n_kt) / (n_f + 1)
                             for i in range(n_f)]
                fq = list(fillers)
                av_i = 0

                def maybe_fill():
                    nonlocal av_i
                    av_i += 1
                    while fq and sched[n_f - len(fq)] <= av_i:
                        fq.pop(0)()

                def emit_mask_exp(t, psc3, et3):
                    """exp + causal masking for one k-tile given [2, nq]
                    views of its score PSUM and exp output. Columns below
                    the diagonal tile's first valid q (P*s) are never
                    computed, exp'd, or streamed by AV — no memset needed."""
                    s = t - q0 // P
                    if s < 0:      # fully unmasked tile
                        nc.scalar.activation(et3[:, :, :nq], psc3[:, :, :nq],
                                             Exp)
                    else:          # diagonal-crossing tile
                        nc.scalar.activation(
                            et3[:, :, P * s:nq], psc3[:, :, P * s:nq], Exp)
                        nc.vector.tensor_tensor(
                            out=et3[:, :, P * s:P * (s + 1)],
                            in0=et3[:, :, P * s:P * (s + 1)],
                            in1=tri[:, None, :].to_broadcast([P, 2, P]),
                            op=MUL,
                        )

                for hp in range(2):
                    # one single-bank PSUM tile per head (g): the next hp's
                    # g-accumulation only waits on THIS g's normalize, and
                    # the two normalizes pipeline.
                    pza = zpp.tile([P, QB], dt.float32, tag="z", name="pza")
                    pzb = zpp.tile([P, QB], dt.float32, tag="z", name="pzb")
                    pz = (pza, pzb)
                    prev = None

                    # Diagonal-crossing tiles (s = t - q0//P >= 0) only
                    # need q-columns >= P*s: trimming the score and AV
                    # matmuls to [c0, nq) saves ~15k fully-masked PE
                    # columns across the schedule (and kills the et
                    # memsets that used to zero them for AV).
                    def emit_av(t, et, tp=None):
                        c0 = max(0, t - q0 // P) * P
                        for g in range(2):
                            h = 2 * hp + g
                            rhs = (et[:, g, c0:nq] if tp is None
                                   else et[:, g, tp, c0:nq])
                            nc.tensor.matmul(
                                pz[g][:, c0:nq],
                                lhsT=vsb[:, t, P * h:P * (h + 1)],
                                rhs=rhs,
                                start=(t == 0), stop=(t == n_kt - 1),
                                skip_group_check=True,
                            )

                    def emit_score(t, out2):
                        c0 = max(0, t - q0 // P) * P
                        for g in range(2):
                            b0 = 64 * g
                            nc.tensor.matmul(
                                out2[:, g, c0:nq],
                                lhsT=kT[b0:b0 + 64, hp, P * t:P * (t + 1)],
                                rhs=qT[b0:b0 + 64, hp, q0 + c0:q0 + nq],
                                start=True, stop=True,
                                tile_position=(b0, 0),
                            )

                    if nq == QB:
                        for t in range(n_kt):
                            psc = scp.tile([P, 2, QB], dt.float32, tag="sc")
                            emit_score(t, psc)
                            et = ep.tile([P, 2, QB], dt.bfloat16, tag="et")
                            emit_mask_exp(t, psc, et)
                            if prev is not None:
                                emit_av(*prev)
                                maybe_fill()
                            prev = (t, et)
                        emit_av(*prev)
                        maybe_fill()
                    else:
                        # 256-wide half blocks: two k-tiles share one PSUM
                        # tile and (when both are unmasked) one exp call.
                        # Layout [g, tp, 256] keeps each head pair's scores
                        # in its own bank (the two row-group matmuls run
                        # concurrently and must not share a PSUM bank).
                        for pr in range(n_kt // 2):
                            t0, t1 = 2 * pr, 2 * pr + 1
                            psc = scp.tile([P, 2, 2, QB // 2], dt.float32,
                                           tag="sc", name="pscp")
                            emit_score(t0, psc[:, :, 0])
                            emit_score(t1, psc[:, :, 1])
                            et = ep.tile([P, 2, 2, QB // 2], dt.bfloat16,
                                         tag="et", name="etp")
                            if t1 - q0 // P < 0:   # both unmasked: one exp
                                nc.scalar.activation(et[:], psc[:], Exp)
                            else:
                                emit_mask_exp(t0, psc[:, :, 0], et[:, :, 0])
                                emit_mask_exp(t1, psc[:, :, 1], et[:, :, 1])
                            if prev is not None:
                                for pt in prev[0]:
                                    emit_av(pt, prev[1], tp=pt % 2)
                                maybe_fill()
                                maybe_fill()
                            prev = ((t0, t1), et)
                        for pt in prev[0]:
                            emit_av(pt, prev[1], tp=pt % 2)
                        maybe_fill()
                        maybe_fill()
                    if hp == 1:
                        while fq:       # safety net; sched places everything
                            fq.pop(0)()
                    # normalize: rows 0-63 of each pz hold the denominator
                    # (replicated by the PE), rows 64-127 the unnormalized z;
                    # fast reciprocal + multiply straight from PSUM, per head
                    # so the banks free one by one.
                    rcp = rp.tile([DH, 2, QB], dt.float32, tag="rcp")
                    for g in range(2):
                        nc.vector.reciprocal_approx_fast(
                            out=rcp[:, g, :nq], in_=pz[g][0:DH, :nq])
                        nc.vector.tensor_tensor(
                            out=zT[64 * g:64 * (g + 1), hp, j,
                                   zq0:zq0 + nq],
                            in0=pz[g][DH:P, :nq],
                            in1=rcp[:, g, :nq],
                            op=MUL,
                        )

            # ---------- main schedule ----------
            # Only what stream-0's head-pair 0 needs goes up front: the m0
            # halves of q(0)/k(0) (two DMA-paced accumulation chains,
            # interleaved per k-chunk) and v-tiles 0-1. The m1 halves (only
            # needed by head-pair 1, ~10us later) and v-tiles 2-3 ride as
            # stream-0 fillers during head-pair 0 — attention starts ~8us
            # earlier than with the full qkv block up front.
            ps_q = scp.tile([P, 2, QB], dt.float32, tag="sc", name="ps_q")
            ps_k = scp.tile([P, 2, QB], dt.float32, tag="sc", name="ps_k")
            for t in range(8):
                for w_sb, ps in ((wq, ps_q), (wk, ps_k)):
                    nc.tensor.matmul(
                        ps[:, 0, :],
                        lhsT=w_sb[:, t, 0:P],
                        rhs=xT[:, t, 0:QB],
                        start=(t == 0), stop=(t == 7),
                        skip_group_check=True,
                    )
            for b_sb, ps, dst in ((bq, ps_q, qT), (bk, ps_k, kT)):
                nc.vector.tensor_tensor(
                    out=dst[:, 0, 0:QB],
                    in0=ps[:, 0, :],
                    in1=b_sb[:, 0, None].to_broadcast([P, QB]),
                    op=ADD,
                )
            # the m1 halves are PE-ready as soon as the m0 chains drain
            # (wq/wk fully landed) and exactly fill the wait for wv
            emit_qk_half(0, "q", 1)
            emit_qk_half(0, "k", 1)
            for jt in range(2):
                emit_v_tile(jt)

            def qkv_chunks(jb):
                return [
                    lambda jb=jb: emit_qk_half(jb, "q", 0),
                    lambda jb=jb: emit_qk_half(jb, "q", 1),
                    lambda jb=jb: emit_qk_half(jb, "k", 0),
                    lambda jb=jb: emit_qk_half(jb, "k", 1),
                ] + [lambda jt=jt: emit_v_tile(jt) for jt in range(4 * jb,
                                                                  4 * jb + 4)]

            # v2/v3 + the m1 halves MUST fire during head-pair 0 (sched
            # points 1-4): head-pair 1's scores depend on kT-m1, and a
            # filler left to the post-emission flush would sit BEHIND them
            # in the PE queue (deadlock).
            attention_stream(0, 0, QB, 4, 0,
                             [lambda: emit_v_tile(2),
                              lambda: emit_v_tile(3)]
                             + qkv_chunks(1),
                             sched=[1, 1, 2, 3, 4, 5, 6, 7, 8, 8])
            emit_a2a(0)
            # deferred low-priority loads: by now (~40us) the startup DMA
            # crunch is over and the HBM port is free
            with tc.tile_wait_until(0.04):
                nc.sync.dma_start(xT[:, :, 2 * QB:3 * QB],
                                  xT_v[:, :, 2 * QB:3 * QB])
                nc.gpsimd.dma_start(xT[:, :, 3 * QB:4 * QB],
                                    xT_v[:, :, 3 * QB:4 * QB])
                nc.sync.dma_start(
                    wo[:], wo_d.ap().rearrange("(o p) f -> p o f", p=P))
                nc.gpsimd.dma_start(bo[:],
                                    bo_d.ap()[0:1, :].to_broadcast([P, D]))
            attention_stream(1, QB, QB, 8, 0, qkv_chunks(2))
            emit_a2a(1)
            # The CC stream can't start before the 8-core mesh barrier ends
            # (~108us), so zrecv(0) lands ~128us at best: final-op fillers
            # must sit well past that — a filler matmul whose zrecv hasn't
            # landed stalls the PE queue head and the whole attention
            # stream behind it (measured: 40us).
            attention_stream(2, 2 * QB, QB, 12, 0,
                             [lambda: emit_qk_half(3, "q", 0),
                              lambda: emit_qk_half(3, "q", 1),
                              lambda: emit_qk_half(3, "k", 0),
                              lambda: emit_qk_half(3, "k", 1)]
                             + [lambda jt=jt: emit_v_tile(jt)
                                for jt in range(12, 16)])
            emit_a2a(2)
            # The CC stream can't start before the slowest core launches
            # (random 0-90us skew): zrecv(j) availability is only reliable
            # ~2 streams after a2a(j)'s trigger, so final-op fillers sit
            # late (a filler matmul whose zrecv hasn't landed stalls the
            # PE queue head and the whole attention stream behind it).
            # final_op(0) sits LATE (stream 3b) and (1) in the tail: on
            # bad-skew runs zrecv(0) can land as late as ~150us, and a
            # stalled filler delays this core's own exchange triggers,
            # cascading the stagger penalty to every peer (measured 25us).
            attention_stream(3, 3 * QB, 256, 14, 0, [])
            attention_stream(3, 3 * QB + 256, 256, 16, 256,
                             [lambda: emit_final_op(0)],
                             sched=[8])
            # block-3 half-exchanges: et=0 needs only head-pair 0's
            # normalize of stream 3b and fires ~half a stream early
            emit_a2a3(0)
            emit_a2a3(1)
            emit_tail()

    nc.compile()
    return nc


def _get_nc():
    if "nc" not in _NC_CACHE:
        _NC_CACHE["nc"] = build_kernel()
    return _NC_CACHE["nc"]


def make_in_maps(normalized_resid_pre, W_Q, W_K, W_V, W_O, b_Q, b_K, b_V, b_O):
    x = np.asarray(normalized_resid_pre, dtype=F32)
    W_Q = np.asarray(W_Q, F32); W_K = np.asarray(W_K, F32)
    W_V = np.asarray(W_V, F32); W_O = np.asarray(W_O, F32)
    b_Q = np.asarray(b_Q, F32); b_K = np.asarray(b_K, F32)
    b_V = np.asarray(b_V, F32); b_O = np.asarray(b_O, F32)

    sq = 1.0 / (D * np.sqrt(DH))            # folded into W_Q / b_Q
    so = 1.0 / (np.sqrt(D) * np.sqrt(DH))   # folded into W_O

    # full scaled W_O for every core, flat [1024, 1024]: row = global z-dim
    # 64h + e = 256*hg + 128*et + p (the kernel-side rearrange chunks it)
    wo_c = np.ascontiguousarray((W_O * so).reshape(D, D).astype(BF16))
    tri = np.triu(np.ones((P, P), dtype=F32)).astype(BF16)  # tri[k,q]=1 iff k<=q
    bo_b = b_O.reshape(1, D).astype(F32)

    in_maps = []
    for c in CORE_IDS:
        b = c // 4
        hg = c % 4
        hs = slice(HC * hg, HC * (hg + 1))
        xT_b = np.ascontiguousarray(x[b].T.astype(BF16))          # [D, S]
        wq_c = np.ascontiguousarray(
            (W_Q[hs] * sq).transpose(1, 0, 2).reshape(D, E).astype(BF16))
        wk_c = np.ascontiguousarray(
            W_K[hs].transpose(1, 0, 2).reshape(D, E).astype(BF16))
        wv_c = np.ascontiguousarray(
            W_V[hs].transpose(1, 0, 2).reshape(D, E).astype(BF16))
        bq_c = np.ascontiguousarray(
            (b_Q[hs] * sq).reshape(E).reshape(2, P).T).astype(F32)  # [P, 2]
        bk_c = np.ascontiguousarray(
            b_K[hs].reshape(E).reshape(2, P).T).astype(F32)
        bv_c = np.ascontiguousarray(
            np.broadcast_to(b_V[hs].reshape(E), (P, E))).astype(F32)
        in_maps.append({
            "xT": xT_b, "wq": wq_c, "wk": wk_c, "wv": wv_c, "wo": wo_c,
            "bq": bq_c, "bk": bk_c, "bv": bv_c, "bo": bo_b, "tri": tri,
        })
    return in_maps


def assemble_out(results):
    # core c, out row 128j + 64b + r  ->  (batch b, seq 512j + 64c + r)
    out = np.empty((B, S, D), dtype=F32)
    for c in CORE_IDS:
        r = results[c]["out"].astype(F32)
        for j in range(4):
            for b in range(B):
                out[b, QB * j + 64 * c:QB * j + 64 * (c + 1), :] = \
                    r[P * j + 64 * b:P * j + 64 * (b + 1)]
    return out


def _ensure_trace_support():
    """If profiling is requested, make sure the axon NTFF hook shim exists
    (this container's antenv package lacks axon_hooks)."""
    try:
        import types
        import antenv

        if "antenv.axon_hooks" not in sys.modules:
            mod = types.ModuleType("antenv.axon_hooks")
            hook = [None]
            mod.set_axon_ntff_profile_hook = lambda h: hook.__setitem__(0, h)
            mod.get_axon_ntff_profile_hook = lambda: hook[0]
            sys.modules["antenv.axon_hooks"] = mod
            antenv.axon_hooks = mod
            from trn_agent_boot.trn_boot import _ntff_profile_via_ctypes

            mod.set_axon_ntff_profile_hook(
                _ntff_profile_via_ctypes("/opt/axon/libaxon_pjrt.so"))
    except Exception:
        pass


def kernel(**inputs):
    from concourse.bass_utils import run_bass_kernel_spmd

    _ensure_trace_support()
    nc = _get_nc()
    in_maps = make_in_maps(**inputs)
    trace = bool(int(os.environ.get("BASS_KERNEL_TRACE", "0")))
    res = run_bass_kernel_spmd(nc, in_maps, CORE_IDS, trace=trace)
    _NC_CACHE["last_result"] = res
    return assemble_out(res.results)

